# revision 49
# baseline (speedup 1.0000x reference)
"""Trainium2 Bass kernel for MultiHeadAttention (RMSNorm + MHA + residual).

Reference computation (B=2, S=2048, D=1024, H=16):
    xn = x * rsqrt(mean(x^2, -1) + 1e-12) * gamma
    q/k/v = (xn @ W{q,k,v}.T) split into heads
    attn  = softmax(q k^T / sqrt(64)) v          (mask is zeros)
    out   = xn + (attn @ Wo.T)

Sharding: tensor-parallel over heads (2 heads/core on 8 cores) for
QKV/scores/softmax/attn-V, then an AllToAll switches to token sharding
for the output projection + residual. Token ownership is STRIPED:
core c owns tokens {g*512 + c*64 + r} so that block g's attention
output provides a 64-token stripe of EVERY core's share -- the AllToAll
is split into 8 chunked collectives (one per q-block) that overlap the
remaining attention compute, and the output projection overlaps too.

Key differences vs the v1 baseline:
  * rstd is computed from feature-major x only: sum-of-squares via an
    all-ones stationary matmul (replicates ssq across all psum
    partitions -- no DRAM broadcast round-trip, which cost ~400us),
    then 1/x via the fast DVE reciprocal and sqrt on ACT.
  * Q/K/V are projected from RAW x and scaled by rstd afterwards
    (linearity), removing the xn materialization entirely.
  * Scores psum (2 banks) and attnV accumulators are arranged so the
    scalar-engine exp (the critical resource: 128 x ~1.15us) overlaps
    matmuls via double-buffered score psum.
  * Z normalization uses reciprocal_approx_fast (~5x faster than DVE
    reciprocal; 18 bits is plenty for a bf16 result).
  * gamma is folded host-side into Wq/Wk/Wv and into the residual rows.
  * per-token rstd for the residual path rides along the A2A payload
    as a 129th feature row.
"""

import numpy as np
import ml_dtypes

import concourse.bacc as bacc
import concourse.mybir as mybir
import concourse.tile as tile
from concourse.bass_utils import run_bass_kernel_spmd
from concourse.masks import make_identity

F32 = mybir.dt.float32
BF16 = mybir.dt.bfloat16
FP8 = mybir.dt.float8e4
AF = mybir.ActivationFunctionType
DROW = mybir.MatmulPerfMode.DoubleRow
W8SCALE = 64.0  # host pre-scale on Wq/Wk so fp8 e4m3 entries are ~O(1)

NCORES = 8
D = 1024
H = 16
DH = 64            # head dim
HPC = H // NCORES  # heads per core
FPC = HPC * DH     # attn features per core


def build(B=2, S=2048, debug_dump=False):
    TOK = B * S
    IC = D // 128        # input-feature chunks
    TG = TOK // 512      # 512-token groups
    TPC = TOK // NCORES  # tokens per core
    LT = TPC // 128      # phase-E token tiles per core
    KT = S // 128        # key tiles per batch
    QCH = 512            # q-block size
    NBLK = TOK // QCH    # attention q-blocks (= A2A chunks)
    NT = TOK // 128
    assert TPC == 512 and NBLK == 8

    nc = bacc.Bacc("TRN2", target_bir_lowering=False, debug=False,
                   num_devices=NCORES)
    xt_d = nc.dram_tensor("xt", [D, TOK], BF16, kind="ExternalInput")
    xt8_d = nc.dram_tensor("xt8", [D, TOK], FP8, kind="ExternalInput")
    xres_d = nc.dram_tensor("xres", [TPC, D], F32, kind="ExternalInput")
    wq_d = nc.dram_tensor("wq", [D, FPC], FP8, kind="ExternalInput")
    wk_d = nc.dram_tensor("wk", [D, FPC], FP8, kind="ExternalInput")
    wv_d = nc.dram_tensor("wv", [D, FPC], BF16, kind="ExternalInput")
    wo_d = nc.dram_tensor("wo", [D, D], BF16, kind="ExternalInput")
    out_d = nc.dram_tensor("out", [TPC, D], F32, kind="ExternalOutput")
    dbg_d = (nc.dram_tensor("dbg", [512, 1024], BF16, kind="ExternalOutput")
             if debug_dump else None)

    with tile.TileContext(nc) as tc:
        with (
            tc.tile_pool(name="sb", bufs=1) as sb,
            tc.tile_pool(name="dram", bufs=1, space="DRAM") as dpool,
        ):
            # per-chunk A2A bounce buffers (separate tiles keep the
            # dependency tracking per-chunk)
            bin_g = [dpool.tile([NCORES, FPC + 1, 64], BF16, name=f"bin{g}")
                     for g in range(NBLK)]
            bout_g = [dpool.tile([NCORES, FPC + 1, 64], BF16, name=f"bout{g}")
                      for g in range(NBLK)]
            # warmup collective: absorbs the first-collective setup cost
            # (ring/credit init) during phase A/B while CC cores are idle.
            # Same shape as a real chunk so the rings are sized right.
            warm_in = dpool.tile([NCORES, FPC + 1, 64], BF16, name="warmin")
            warm_out = dpool.tile([NCORES, FPC + 1, 64], BF16, name="warmout")
            nc.gpsimd.collective_compute(
                "AllToAll", mybir.AluOpType.bypass,
                replica_groups=[list(range(NCORES))],
                ins=[warm_in[:].opt()], outs=[warm_out[:].opt()])

            # ---- persistent weights / constants ----
            # DMA order matters: the sync queue is serial, so x (needed
            # first) goes ahead of Wq/Wk/Wv; Wo (phase E only) goes last.
            # Wq/Wk are fp8 in DoubleRow layout [Ki, dc, Ko=2, M]: logical
            # input feature = dc*256 + ko*128 + ki.
            wq_sb = sb.tile([128, IC // 2, 2, FPC], FP8, tag="wq")
            wk_sb = sb.tile([128, IC // 2, 2, FPC], FP8, tag="wk")
            wv_sb = sb.tile([128, IC, FPC], BF16, tag="wv")
            wo_sb = sb.tile([128, IC, D], BF16, tag="wo")
            ident = sb.tile([128, 128], BF16, tag="ident")
            make_identity(nc, ident[:])
            ones_sb = sb.tile([128, 128], BF16, tag="ones")
            nc.vector.memset(ones_sb[:], 1.0)

            QT = sb.tile([128, TOK], BF16, tag="qt")
            KTt = sb.tile([128, TOK], BF16, tag="kt")
            rstdB = sb.tile([128, TOK], BF16, tag="rstdB")
            # V (token-major): cols 0..63 per head are ONES (Z lands on
            # psum partitions 0..63 where reciprocal_approx_fast can read
            # it -- the custom DVE op mishandles partition offsets), cols
            # 64..127 are V so attn lands on partitions 64..127.
            v_sb = sb.tile([128, NT, HPC, 128], BF16, tag="v")
            nc.vector.memset(v_sb[:, :, :, 0:DH], 1.0)

            # batch-1's fp8 x is held through phase C (its Q/K projections
            # run interleaved with batch-0's attention)
            xtg8h = [sb.tile([128, IC // 2, 2, 512], FP8, tag=f"x8h{j}",
                             name=f"x8h{j}") for j in range(4)]

            # helper: Q/K/V projections + V transpose for one token group.
            # psum tiles come from `mk(kind)` so phase C can reuse its own
            # pool tags for the batch-1 groups.
            def project_tg(tg, xtg8_t, xtg_t, mk):
                sl = slice(tg * 512, (tg + 1) * 512)
                pq = mk(0, f"pq{tg}")
                pk = mk(1, f"pk{tg}")
                for w_sb, ps_t in ((wq_sb, pq), (wk_sb, pk)):
                    for dc in range(IC // 2):
                        nc.tensor.matmul(
                            ps_t[:], w_sb[:, dc, :, :], xtg8_t[:, dc, :, :],
                            start=(dc == 0), stop=(dc == IC // 2 - 1),
                            perf_mode=DROW)
                nc.vector.tensor_mul(QT[:, sl], pq[:], rstdB[:, sl])
                nc.vector.tensor_mul(KTt[:, sl], pk[:], rstdB[:, sl])
                pv = mk(0, f"pv{tg}")
                for ic in range(IC):
                    nc.tensor.matmul(
                        pv[:], wv_sb[:, ic, :], xtg_t[:, ic, :],
                        start=(ic == 0), stop=(ic == IC - 1))
                vt_t = sb.tile([128, 512], BF16, tag="vt", bufs=2,
                               name=f"vt{tg}")
                nc.vector.tensor_mul(vt_t[:], pv[:], rstdB[:, sl])
                ptr4 = mk(2, f"ptr{tg}")
                for j in range(4):
                    nc.tensor.transpose(
                        ptr4[:, j, :], vt_t[:, j * 128:(j + 1) * 128],
                        ident[:])
                nc.vector.tensor_copy(
                    v_sb[:, tg * 4:(tg + 1) * 4, :, DH:128],
                    ptr4[:].rearrange("p j (h f) -> p j h f", h=HPC))

            # ---- part 1: rstd for ALL tokens (keeps every Sqrt ahead of
            # the exp table load), full projections for batch 0 only;
            # batch 1's projections overlap batch-0 attention below ----
            with tc.tile_pool(name="psAB", bufs=1, space="PSUM") as psB:
                def mkAB(kind, name):
                    if kind == 2:
                        return psB.tile([128, 4, 128], BF16, tag="ptr",
                                        bufs=2, name=name)
                    return psB.tile([128, 512], F32, tag="pqk", bufs=3,
                                    name=name)

                for tg in (0, 4, 1, 5, 2, 6, 3, 7):
                    sl = slice(tg * 512, (tg + 1) * 512)
                    if tg < 4:
                        # batch 0 needs bf16 x for the V projection
                        xtg = sb.tile([128, IC, 512], BF16, tag="xtg",
                                      bufs=3, name=f"xtg{tg}")
                        nc.sync.dma_start(
                            xtg[:],
                            xt_d[:, sl].rearrange("(ic p) t -> p ic t",
                                                  p=128))
                        xtg8 = sb.tile([128, IC // 2, 2, 512], FP8,
                                       tag="xtg8", bufs=2, name=f"xtg8{tg}")
                    else:
                        # batch 1: fp8 copy only (x^2 for ssq comes from
                        # it too -- saves 1MB of DMA per group)
                        xtg = None
                        xtg8 = xtg8h[tg - 4]
                    nc.sync.dma_start(
                        xtg8[:],
                        xt8_d[:, sl].rearrange("(dc ko p) t -> p dc ko t",
                                               p=128, ko=2))
                    if tg == 0:
                        for w_sb, w_d in ((wq_sb, wq_d), (wk_sb, wk_d)):
                            nc.sync.dma_start(
                                w_sb[:],
                                w_d[:].rearrange("(dc ko p) f -> p dc ko f",
                                                 p=128, ko=2))
                        nc.sync.dma_start(
                            wv_sb[:],
                            wv_d[:].rearrange("(ic p) f -> p ic f", p=128))
                    # ssq chain first so rstd is ready by the time the
                    # projections drain (the tensor queue is FIFO). Part 1
                    # is vector-bound, so the squares are split between the
                    # DVE and the otherwise-idle (but ~3x slower) GpSimd.
                    xsq = sb.tile([128, IC, 512], BF16, tag="xsq", bufs=2)
                    if xtg is not None:
                        nc.vector.tensor_mul(xsq[:, 0:5, :], xtg[:, 0:5, :],
                                             xtg[:, 0:5, :])
                        nc.gpsimd.tensor_mul(xsq[:, 5:IC, :],
                                             xtg[:, 5:IC, :],
                                             xtg[:, 5:IC, :])
                    else:
                        x8v = xtg8[:].rearrange("p dc ko t -> p (dc ko) t")
                        nc.vector.tensor_mul(xsq[:, 0:5, :], x8v[:, 0:5, :],
                                             x8v[:, 0:5, :])
                        nc.gpsimd.tensor_mul(xsq[:, 5:IC, :],
                                             x8v[:, 5:IC, :],
                                             x8v[:, 5:IC, :])
                    ssq = psB.tile([128, 512], F32, tag="ssq", bufs=2,
                                   name=f"ssq{tg}")
                    for ic in range(IC):
                        nc.tensor.matmul(
                            ssq[:], ones_sb[:], xsq[:, ic, :],
                            start=(ic == 0), stop=(ic == IC - 1))
                    inv_t = sb.tile([128, 512], F32, tag="inv", bufs=2)
                    nc.vector.reciprocal_approx_fast(inv_t[:], ssq[:])
                    # rstd = sqrt(D / ssq)
                    nc.scalar.activation(rstdB[:, sl], inv_t[:], AF.Sqrt,
                                         scale=float(D))
                    if tg < 4:
                        project_tg(tg, xtg8, xtg, mkAB)

            if debug_dump:
                nc.sync.dma_start(dbg_d[0:128, :], QT[:, 0:1024])
                nc.sync.dma_start(dbg_d[128:256, :], KTt[:, 0:1024])

            # ---- phase C+E pool: scores(4) + attnV(2) + outproj(2) banks --
            with tc.tile_pool(name="psC", bufs=1, space="PSUM") as psC:

                # phase E for one or two A2A chunks: output projection +
                # residual for the 64*nch tokens this core owns. Chunk
                # PAIRS (nch=2) fill the PE stationary (M=128) and halve
                # the matmul count that competes with attention; the final
                # chunks run alone (nch=1) to minimize the post-C tail.
                def phase_e(g0, nch, anchor=False):
                    # anchor=True allocates the A2A-reading tiles from
                    # phase C's rotating tags ("e"/"an", same byte sizes):
                    # the buffer-reuse WAR dependency pins this chunk's
                    # schedule position to the C block it was emitted
                    # after -- the scheduler's (optimistic) collective
                    # latency model then can't pull it early and wedge
                    # the queues behind a slow AllToAll.
                    t0 = g0 * 64
                    W = 64 * nch
                    if anchor:
                        at = sb.tile([128, NCORES, 128], BF16, tag="rz",
                                     bufs=2, name=f"at{g0}")
                        rstdE = sb.tile([64, QCH], BF16, tag="an", bufs=6,
                                        name=f"rse{g0}")
                    else:
                        at = sb.tile([128, NCORES, 128], BF16, tag="at",
                                     bufs=2, name=f"at{g0}")
                        rstdE = sb.tile([64, QCH], BF16, tag="rse", bufs=2,
                                        name=f"rse{g0}")
                    for j in range(nch):
                        nc.sync.dma_start(
                            at[:, :, j * 64:(j + 1) * 64],
                            bout_g[g0 + j][:, 0:FPC, :]
                            .rearrange("s f r -> f s r"))
                        nc.sync.dma_start(
                            rstdE[0:64, j:j + 1],
                            bout_g[g0 + j][0:1, FPC:FPC + 1, :]
                            .rearrange("s o r -> r (s o)"))
                    rstdE_f = sb.tile([128, 1], F32, tag="rsef", bufs=2,
                                      name=f"rsef{g0}")
                    for j in range(nch):
                        nc.vector.tensor_copy(
                            rstdE_f[j * 64:(j + 1) * 64, 0:1],
                            rstdE[0:64, j:j + 1])
                    po = [psC.tile([128, 512], F32, tag=f"po{ng}", bufs=1,
                                   name=f"po{ng}_{g0}")
                          for ng in range(2)]
                    for ng in range(2):
                        for s in range(NCORES):
                            nc.tensor.matmul(
                                po[ng][0:W, :], at[:, s, 0:W],
                                wo_sb[:, s, ng * 512:(ng + 1) * 512],
                                start=(s == 0), stop=(s == NCORES - 1))
                    x_r = sb.tile([128, D], F32, tag="xr", bufs=2,
                                  name=f"xr{g0}")
                    nc.sync.dma_start(x_r[0:W, :], xres_d[t0:t0 + W, :])
                    # xres already carries gamma; xn*gamma = xres * rstd
                    xg = sb.tile([128, D], F32, tag="xg", bufs=2,
                                 name=f"xg{g0}")
                    nc.vector.tensor_scalar_mul(xg[0:W, :], x_r[0:W, :],
                                                rstdE_f[0:W, 0:1])
                    ot = sb.tile([128, D], F32, tag="ot", bufs=2,
                                 name=f"ot{g0}")
                    for ng in range(2):
                        nc.vector.tensor_add(
                            ot[0:W, ng * 512:(ng + 1) * 512],
                            xg[0:W, ng * 512:(ng + 1) * 512], po[ng][0:W, :])
                    nc.sync.dma_start(out_d[t0:t0 + W, :], ot[0:W, :])

                # ---- phase C: attention, one A2A chunk per q-block.
                # phase_e(g) is emitted a fixed 3-block LAG after chunk g's
                # collective is triggered: the static schedule then keeps
                # ~3 blocks (~55us) of attention ahead of any instruction
                # that waits on a collective result, which rides out the
                # run-to-run variance of core-launch skew + A2A latency. --
                E_LAG = 3
                for g in range(NBLK):
                    b, qq = divmod(g, S // QCH)
                    q0 = b * S + qq * QCH
                    pa = [psC.tile([128, QCH], F32, tag=f"pa{h}", bufs=1,
                                   name=f"pa{h}_{g}")
                          for h in range(HPC)]
                    for kt in range(KT):
                        gt = b * KT + kt
                        k0 = b * S + kt * 128
                        p_s = psC.tile([128, HPC * QCH], F32, tag="ps",
                                       bufs=2, name=f"ps_{g}_{kt}")
                        for h in range(HPC):
                            lo = h * DH
                            nc.tensor.matmul(
                                p_s[:, h * QCH:(h + 1) * QCH],
                                KTt[lo:lo + DH, k0:k0 + 128],
                                QT[lo:lo + DH, q0:q0 + QCH],
                                start=True, stop=True)
                        e_t = sb.tile([128, HPC * QCH], BF16, tag="e", bufs=4)
                        # Q and K each carry a host-side W8SCALE factor
                        nc.scalar.activation(e_t[:], p_s[:], AF.Exp,
                                             scale=0.125 / (W8SCALE * W8SCALE))
                        for h in range(HPC):
                            nc.tensor.matmul(
                                pa[h][:], v_sb[:, gt, h, :],
                                e_t[:, h * QCH:(h + 1) * QCH],
                                start=(kt == 0), stop=(kt == KT - 1))
                    # normalize by Z (psum rows 0..63, see v_sb layout)
                    # and scatter the 8 x 64-token stripes into the bounce
                    for h in range(HPC):
                        rz = sb.tile([64, QCH], F32, tag="rz", bufs=2)
                        nc.vector.reciprocal_approx_fast(
                            rz[:], pa[h][0:64, :])
                        an = sb.tile([64, QCH], BF16, tag="an", bufs=6)
                        nc.vector.tensor_mul(an[:], pa[h][64:128, :], rz[:])
                        if debug_dump and g == 0 and h == 0:
                            nc.sync.dma_start(dbg_d[256:320, 0:512], an[:])
                            rzb = sb.tile([64, QCH], BF16, tag="rzb")
                            nc.vector.tensor_copy(rzb[:], rz[:])
                            nc.sync.dma_start(dbg_d[320:384, 0:512], rzb[:])
                            pab = sb.tile([64, QCH], BF16, tag="pab")
                            nc.vector.tensor_copy(pab[:], pa[h][64:128, :])
                            nc.sync.dma_start(dbg_d[384:448, 0:512], pab[:])
                            zb = sb.tile([64, QCH], BF16, tag="zb")
                            nc.vector.tensor_copy(zb[:], pa[h][0:64, :])
                            nc.sync.dma_start(dbg_d[448:512, 0:512], zb[:])
                        nc.sync.dma_start(
                            bin_g[g][:, h * DH:(h + 1) * DH, :]
                            .rearrange("s f r -> f s r"),
                            an[:].rearrange("f (s r) -> f s r", s=NCORES))
                    # per-token rstd rides along as feature row 128
                    nc.sync.dma_start(
                        bin_g[g][:, FPC:FPC + 1, :]
                        .rearrange("s o r -> o s r"),
                        rstdB[0:1, g * 512:(g + 1) * 512]
                        .rearrange("o (s r) -> o s r", s=NCORES))
                    nc.gpsimd.collective_compute(
                        "AllToAll", mybir.AluOpType.bypass,
                        replica_groups=[list(range(NCORES))],
                        ins=[bin_g[g][:].opt()],
                        outs=[bout_g[g][:].opt()])
                    if g == 0:
                        nc.sync.dma_start(
                            wo_sb[:],
                            wo_d[:].rearrange("(ic p) f -> p ic f", p=128))
                    # batch-1 projections ride in batch-0's attention slack
                    # (psum borrowed from the phase-E po tags, which are
                    # first needed only after block 5)
                    if g < 4:
                        tg1 = 4 + g
                        xtgv = sb.tile([128, IC, 512], BF16, tag="xtgv",
                                       bufs=2, name=f"xtgv{g}")
                        nc.sync.dma_start(
                            xtgv[:],
                            xt_d[:, tg1 * 512:(tg1 + 1) * 512]
                            .rearrange("(ic p) t -> p ic t", p=128))

                        def mkC(kind, name, _g=g):
                            if kind == 2:
                                return psC.tile([128, 4, 128], BF16,
                                                tag="po1", bufs=1, name=name)
                            return psC.tile([128, 512], F32,
                                            tag=f"po{kind}", bufs=1,
                                            name=name)

                        project_tg(tg1, xtg8h[g], xtgv, mkC)
                    # E schedule: pairs (0,1),(2,3) emitted 4 blocks after
                    # their LAST chunk's collective (the CC stream runs
                    # ~2 blocks behind early on); the rest post-loop,
                    # overlapping the trailing collectives.
                    if g == 5:
                        phase_e(0, 2, anchor=True)
                    elif g == 6:
                        phase_e(2, 2, anchor=True)
                    elif g == 7:
                        phase_e(4, 2, anchor=True)

                phase_e(6, 1)
                phase_e(7, 1)

    nc.compile()
    return nc


_CACHE = {}


def _get_nc(B=2, S=2048):
    key = (B, S)
    if key not in _CACHE:
        _CACHE[key] = build(B, S)
    return _CACHE[key]


def make_in_maps(x, Wq, Wk, Wv, Wo, gamma, B, S):
    TOK = B * S
    bf = ml_dtypes.bfloat16
    f8 = ml_dtypes.float8_e4m3fn
    x2d = np.ascontiguousarray(np.asarray(x, np.float32).reshape(TOK, D))
    xT = x2d.T
    xt = np.ascontiguousarray(xT.astype(bf))
    xt8 = np.ascontiguousarray(xT.astype(f8))
    gam = np.asarray(gamma, np.float32).reshape(D)
    woT = np.ascontiguousarray(np.asarray(Wo, np.float32).T.astype(bf))
    # residual rows carry gamma already, striped: core c owns tokens
    # {g*512 + c*64 + r}
    xg_res = (x2d * gam[None, :]).reshape(NCORES, NCORES, 64, D)
    in_maps = []
    for c in range(NCORES):
        fs = slice(c * FPC, (c + 1) * FPC)
        m = {
            "xt": xt,
            "xt8": xt8,
            "xres": np.ascontiguousarray(
                xg_res[:, c].reshape(TOK // NCORES, D)),
            "wo": woT,
        }
        for name, W, dt, sc in (("wq", Wq, f8, W8SCALE),
                                ("wk", Wk, f8, W8SCALE),
                                ("wv", Wv, bf, 1.0)):
            Wc = np.asarray(W, np.float32)[fs, :] * gam[None, :] * sc
            m[name] = np.ascontiguousarray(Wc.T.astype(dt))
        in_maps.append(m)
    return in_maps


def kernel(x, attn_mask, Wq, Wk, Wv, Wo, gamma, _trace=False):
    B, S, _ = np.asarray(x).shape
    nc = _get_nc(B, S)
    in_maps = make_in_maps(x, Wq, Wk, Wv, Wo, gamma, B, S)
    res = run_bass_kernel_spmd(nc, in_maps, core_ids=list(range(NCORES)),
                               trace=_trace)
    # core c's rows are (g, r) stripes: out[g*512 + c*64 + r] = res[c][g*64+r]
    allres = np.stack([res.results[c]["out"] for c in range(NCORES)], axis=0)
    out = allres.reshape(NCORES, NCORES, 64, D).transpose(1, 0, 2, 3)
    out = out.reshape(B, S, D).astype(np.float32)
    if _trace:
        kernel.last_results = res
    return out
